# revision 51
# baseline (speedup 1.0000x reference)
"""Causal multi-head self-attention on 8 Trainium2 NeuronCores.

Problem (hardcoded): x [4, 2048, 1024] fp32, w_qkv [3072, 1024], w_out
[1024, 1024], token_positions [2048] int32; H=16 heads, Dh=64, RoPE
(interleaved pairs, theta=10000), causal softmax, output projection.

Sharding: 8 cores = 4 batches x 2 head-groups (8 heads each). Each core
computes qkv projection for its heads, RoPE, causal attention, and a
partial output projection over its 512 y-features. Host sums the two
partial projections per batch and transposes back.

Single fused pipeline, chunk-major, engineered to keep the PE streaming
continuously (the cost floor is matmul moving-rows; everything else has
slack):
  - all matmul operands f16 (1 cycle/row at any tile size; psum f32)
  - per seq-chunk c: project qkv for chunk c+1 and run attention for
    chunk c *interleaved at matmul granularity*, so the ScalarE exp
    (the attention pacer) always overlaps spare PE work
  - attention t-loop is software-pipelined: scores(t+1) issue before
    PV(t), which waits on exp(t)
  - out-projection (chunk pc) quanta are scheduled as late fillers
    (during attention of later chunks) to cover the exp-bound stretch
  - RoPE: q_rot = C*q + S'*shuffle(q) on DVE+Pool, off the PE path
  - v tiles carry an appended ones column so PV also yields softmax
    denominators; divide via reciprocal + partition_broadcast
"""

import math
from collections import deque

import numpy as np

_ROPE_BUFS = 3
_BLOCK_FILL = 1
_NORM_STYLE = 2     # divide reads y straight from PSUM

import concourse.bacc as bacc
import concourse.mybir as mybir
import concourse.tile as tile
from concourse.bass_utils import run_bass_kernel_spmd

F32 = mybir.dt.float32
F16 = mybir.dt.float16

B, S, D = 4, 2048, 1024
H = 16
DH = 64
H_CORE = 8          # heads per core
N_CORES = 8
ROPE_THETA = 10000.0

CH = 512            # seq chunk (free dim of most matmuls)
N_CHUNKS = S // CH          # 4
N_STILES = S // 128         # 16
N_DTILES = D // 128         # 8
SWAP_MASK = [i ^ 1 for i in range(32)]

_EXP = mybir.ActivationFunctionType.Exp


def build_nc():
    """Build + compile the SPMD single-core program (identical on all cores)."""
    nc = bacc.Bacc("TRN2", target_bir_lowering=False, debug=False)

    xT = nc.dram_tensor("xT", [D, S], F16, kind="ExternalInput").ap()
    # [d, f] with f = [q-heads (512) | k-heads (512)] for this core's 8 heads
    wqkT = nc.dram_tensor("wqkT", [D, 2 * H_CORE * DH], F16, kind="ExternalInput").ap()
    wvT = nc.dram_tensor("wvT", [D, H_CORE * DH], F16, kind="ExternalInput").ap()
    woT = nc.dram_tensor("woT", [H_CORE * DH, D], F16, kind="ExternalInput").ap()
    cosT = nc.dram_tensor("cosT", [128, S], F16, kind="ExternalInput").ap()
    sinT = nc.dram_tensor("sinT", [128, S], F16, kind="ExternalInput").ap()
    # [tri x4]: tri[i, j] = 1 if i <= j else 0
    trimask = nc.dram_tensor("trimask", [128, 512], F16, kind="ExternalInput").ap()
    outT = nc.dram_tensor("outT", [D, S], F32, kind="ExternalOutput").ap()

    with tile.TileContext(nc) as tc:
        _build_body(nc, tc, xT, wqkT, wvT, woT, cosT, sinT, trimask, outT)
    nc.compile()
    return nc


def _build_body(nc, tc, xT, wqkT, wvT, woT, cosT, sinT, trimask, outT):
    with tc.tile_pool(name="persist", bufs=1) as persist, \
         tc.tile_pool(name="xch", bufs=2) as xch_pool, \
         tc.tile_pool(name="rope", bufs=_ROPE_BUFS) as rope_pool, \
         tc.tile_pool(name="et", bufs=6) as et_pool, \
         tc.tile_pool(name="sm", bufs=2) as sm_pool, \
         tc.tile_pool(name="osb", bufs=6) as osb_pool, \
         tc.tile_pool(name="ps", bufs=2, space="PSUM") as ps_pool:

        cos_sb = persist.tile([128, S], F16, tag="cos", name="cos_sb")
        sin_sb = persist.tile([128, S], F16, tag="sin", name="sin_sb")
        tri_sb = persist.tile([128, 512], F16, tag="tri", name="tri_sb")

        wqk_all = persist.tile([128, N_DTILES * 1024], F16, tag="wqk", name="wqk_all")
        wqk_sb = [wqk_all[:, 1024 * dt:1024 * (dt + 1)] for dt in range(N_DTILES)]
        wv_all = persist.tile([128, N_DTILES * 512], F16, tag="wv", name="wv_all")
        wv_sb = [wv_all[:, 512 * dt:512 * (dt + 1)] for dt in range(N_DTILES)]
        wo_all = persist.tile([128, 4 * 1024], F16, tag="wo", name="wo_all")
        wo_sb = [wo_all[:, 1024 * dt:1024 * (dt + 1)] for dt in range(4)]

        # k for all chunks; q double-buffered by chunk parity
        k_rot = [[persist.tile([128, CH], F16, tag=f"k{hp}_{c}", name=f"krot{hp}_{c}")
                  for c in range(N_CHUNKS)] for hp in range(4)]
        q_rot = [[persist.tile([128, CH], F16, tag=f"q{hp}_{sl}", name=f"qrot{hp}_{sl}")
                  for sl in range(2)] for hp in range(4)]
        v_ext = [persist.tile([128, H_CORE * 65], F16, tag=f"v{st}", name=f"vext{st}")
                 for st in range(N_STILES)]
        yT = [persist.tile([128, S], F16, tag=f"yT{hp}", name=f"yT{hp}")
              for hp in range(4)]

        # junk init first so the PE warmup can start immediately
        junk = persist.tile([128, 128], F16, tag="junk", name="junk_sm")
        nc.vector.memset(junk[:], 1.0)
        junk5 = persist.tile([128, 512], F16, tag="junk5", name="junk5_sm")
        nc.vector.memset(junk5[:], 1.0)
        warm = persist.tile([128, 8], F32, tag="warm", name="warm_sm")

        ones_sm = persist.tile([128, H_CORE], F16, tag="ones1", name="ones_sm")
        nc.vector.memset(ones_sm[:], 1.0)
        for st in range(N_STILES):
            nc.vector.tensor_copy(v_ext[st][:, 64::65], ones_sm[:])

        # ---- prologue DMAs: one consolidated transfer per tensor, in
        #      consumption order (SP.SEQ issues serialize at ~650ns each) ----
        x_ch = {}

        def load_x(c):
            xa = xch_pool.tile([128, N_DTILES * CH], F16, tag="xc", name=f"xch{c}")
            nc.sync.dma_start(
                xa[:].rearrange("p (dt s) -> p dt s", s=CH),
                xT[:, CH * c:CH * (c + 1)].rearrange("(dt p) s -> p dt s", p=128))
            x_ch[c] = [xa[:, CH * dt:CH * (dt + 1)] for dt in range(N_DTILES)]

        # Consumption order; cos gates the first rope's t1 which gates the
        # psum fill-slot rotation, so it loads right after wqk.
        load_x(0)
        for h in range(2):
            nc.sync.dma_start(
                wqk_all[:, 4096 * h:4096 * (h + 1)].rearrange(
                    "p (dt f) -> p dt f", f=1024),
                wqkT[512 * h:512 * (h + 1), :].rearrange(
                    "(dt p) f -> p dt f", p=128))
        nc.sync.dma_start(cos_sb[:], cosT)
        nc.sync.dma_start(sin_sb[:], sinT)
        nc.sync.dma_start(wv_all[:].rearrange("p (dt f) -> p dt f", f=512),
                          wvT.rearrange("(dt p) f -> p dt f", p=128))
        nc.sync.dma_start(tri_sb[:], trimask)
        nc.sync.dma_start(wo_all[:].rearrange("p (dt f) -> p dt f", f=1024),
                          woT.rearrange("(dt p) f -> p dt f", p=128))
        load_x(1)

        # ---- warmup: keep PE busy while the first DMAs land, and ramp
        #      the PE pstate to full clock ----
        for g, (rhs, n) in enumerate(((junk, 8), (junk5, 8), (junk5, 8))):
            ps_w = ps_pool.tile([128, 512], F32, tag="fill", name=f"ps_warm{g}")
            for i in range(n):
                nc.tensor.matmul(ps_w[:, 0:rhs.shape[-1]], junk[:], rhs[:],
                                 start=(i == 0), stop=(i == n - 1))
            nc.vector.tensor_copy(warm[0:1, g:g + 1], ps_w[0:1, 0:1])

        # ---- filler machinery: generators yielding at PE-quantum
        #      boundaries, drained into attention gaps ----
        fillers = deque()
        pending = [0]       # outstanding filler quanta (next() calls left)

        def emit_fill(n):
            while n > 0 and fillers:
                try:
                    next(fillers[0])
                except StopIteration:
                    fillers.popleft()
                n -= 1
                pending[0] = max(0, pending[0] - 1)

        def gen_qk_group(c, ft):
            """QK projection ftile (128 features) + rope. ft 0-3 = q pairs,
            ft 4-7 = k pairs."""
            cs = slice(CH * c, CH * (c + 1))
            dest = q_rot[ft][c % 2] if ft < 4 else k_rot[ft - 4][c]
            ps_qk = ps_pool.tile([128, CH], F32, tag="fill", name=f"psqk{c}_{ft}")
            for dt in range(N_DTILES):
                nc.tensor.matmul(
                    ps_qk[:],
                    wqk_sb[dt][:, 128 * ft:128 * (ft + 1)],
                    x_ch[c][dt][:],
                    start=(dt == 0), stop=(dt == N_DTILES - 1),
                )
                if dt == 3:
                    yield
            yield
            shuf = rope_pool.tile([128, CH], F32, tag="shuf", name=f"shuf{c}_{ft}")
            nc.vector.stream_shuffle(shuf[:], ps_qk[:], SWAP_MASK)
            t1 = rope_pool.tile([128, CH], F32, tag="t1", name=f"t1_{c}_{ft}")
            nc.vector.tensor_mul(t1[:], ps_qk[:], cos_sb[:, cs])
            t2 = rope_pool.tile([128, CH], F32, tag="t2", name=f"t2_{c}_{ft}")
            nc.gpsimd.tensor_mul(t2[:], shuf[:], sin_sb[:, cs])
            nc.gpsimd.tensor_add(dest[:], t1[:], t2[:])

        def gen_v_group(c, stl):
            """V projection for s-tile 4c+stl (natural layout, all 8 heads)."""
            st = 4 * c + stl
            ps_v = ps_pool.tile([128, 512], F32, tag="fill", name=f"psv{st}")
            for dt in range(N_DTILES):
                nc.tensor.matmul(
                    ps_v[:],
                    x_ch[c][dt][:, 128 * stl:128 * (stl + 1)],
                    wv_sb[dt][:],
                    start=(dt == 0), stop=(dt == N_DTILES - 1),
                )
                if dt == 3:
                    yield
            yield
            out_ap = v_ext[st][:, 0:H_CORE * 65].rearrange(
                "p (h e) -> p h e", e=65)[:, :, 0:64]
            in_ap = ps_v[:].rearrange("p (h e) -> p h e", e=64)
            nc.vector.tensor_copy(out_ap, in_ap)

        def gen_p3_group(pc, ot, on_act=False):
            """Out-projection: outT[128*ot:, chunk pc] from yT. Tail groups
            copy psum->sbuf on the (then idle) ScalarE so the fill-slot
            release doesn't queue behind DVE's normalize chain."""
            ps_o = ps_pool.tile([128, CH], F32, tag="fill", name=f"pso{pc}_{ot}")
            for dt in range(4):
                nc.tensor.matmul(
                    ps_o[:],
                    wo_sb[dt][:, 128 * ot:128 * (ot + 1)],
                    yT[dt][:, CH * pc:CH * (pc + 1)],
                    start=(dt == 0), stop=(dt == 3),
                )
                if dt == 1:
                    yield
            osb = osb_pool.tile([128, CH], F32, tag="osb", name=f"osb{pc}_{ot}")
            if on_act:
                nc.scalar.copy(osb[:], ps_o[:])
            else:
                nc.vector.tensor_copy(osb[:], ps_o[:])
            nc.sync.dma_start(
                outT[128 * ot:128 * (ot + 1), CH * pc:CH * (pc + 1)], osb[:])

        def queue_proj(c, v_first=False):
            qk = [gen_qk_group(c, ft) for ft in range(8)]
            v = [gen_v_group(c, stl) for stl in range(4)]
            fillers.extend(v + qk if v_first else qk + v)
            pending[0] += 36    # 12 generators x 3 quanta

        def queue_p3(pc, ots=range(8), on_act=False):
            for ot in ots:
                fillers.append(gen_p3_group(pc, ot, on_act))
            pending[0] += 2 * len(ots)   # 2 quanta per generator

        # ---- attention block: chunk c, head-pair hp; software-pipelined
        #      with fill_n filler quanta injected per t-iter ----
        def emit_att_block(c, hp, fill_rate):
            qt = q_rot[hp][c % 2]
            nt = 4 * c + 4
            facc = [0.0]

            def fill_tick():
                facc[0] += fill_rate
                n = int(facc[0])
                if n:
                    facc[0] -= n
                    emit_fill(n)
            pv0 = ps_pool.tile([65, CH], F32, tag="pv", name=f"pv0_{c}_{hp}")
            pv1 = ps_pool.tile([65, CH], F32, tag="pv", name=f"pv1_{c}_{hp}")

            def scores(t):
                r = t - 4 * c
                coff = 128 * r if r > 0 else 0
                ps_s = ps_pool.tile([128, 2 * CH], F32, tag="ps_s",
                                    name=f"pss{c}_{hp}_{t}")
                kt = k_rot[hp][t // 4]
                ks = slice(128 * (t % 4), 128 * (t % 4 + 1))
                nc.tensor.matmul(
                    ps_s[:, coff:CH],
                    kt[0:64, ks], qt[0:64, coff:CH],
                    start=True, stop=True)
                nc.tensor.matmul(
                    ps_s[:, CH + coff:2 * CH],
                    kt[64:128, ks], qt[64:128, coff:CH],
                    start=True, stop=True)
                return ps_s, coff

            def expmask(t, ps_s, coff):
                et = et_pool.tile([128, 2 * CH], F16, tag="et",
                                  name=f"et{c}_{hp}_{t}")
                src = ps_s[:].rearrange("p (b n) -> p b n", b=2)[:, :, coff:CH]
                dst = et[:].rearrange("p (b n) -> p b n", b=2)[:, :, coff:CH]
                nc.scalar.activation(dst, src, _EXP, scale=1.0 / math.sqrt(DH))
                if t >= 4 * c:
                    dg = et[:].rearrange("p (b n) -> p b n", b=2)[
                        :, :, coff:coff + 128]
                    nc.vector.tensor_mul(
                        dg, dg,
                        tri_sb[:, 0:256].rearrange("p (b n) -> p b n", b=2))
                return et

            def pv_mm(t, et, coff):
                for hl, pv in ((0, pv0), (1, pv1)):
                    hcol = (2 * hp + hl) * 65
                    nc.tensor.matmul(
                        pv[:, coff:CH],
                        v_ext[t][:, hcol:hcol + 65],
                        et[:, CH * hl + coff:CH * hl + CH],
                        start=(t == 0), stop=(t == nt - 1),
                    )

            emit_fill(_BLOCK_FILL)   # cover this block's pipeline-fill bubble
            prev = None
            for t in range(nt):
                ps_s, coff = scores(t)
                if prev is not None:
                    fill_tick()
                    pv_mm(*prev)
                et = expmask(t, ps_s, coff)
                prev = (t, et, coff)
            fill_tick()
            pv_mm(*prev)

            # normalize: y /= softmax denominators (row 64 of pv). One
            # 65-row copy per half releases the psum bank quickly; the
            # reciprocal chain runs from SBUF off the critical path.
            for hl, pv in ((0, pv0), (1, pv1)):
                # the reciprocal's bit-trick needs an SBUF operand starting at
                # partition 0 - stage the denominator row first; the final
                # divide reads y straight from PSUM (plain mul is PSUM-safe)
                sm = sm_pool.tile([1, CH], F32, tag="sm", name=f"smm{c}_{hp}_{hl}")
                nc.vector.tensor_copy(sm[:], pv[64:65, :])
                rc = sm_pool.tile([1, CH], F32, tag="rc", name=f"rc{c}_{hp}_{hl}")
                nc.vector.reciprocal_approx_fast(rc[:], sm[:])
                bc = sm_pool.tile([64, CH], F32, tag="bc", name=f"bc{c}_{hp}_{hl}")
                nc.gpsimd.partition_broadcast(bc[:], rc[:])
                if _NORM_STYLE == 1:
                    ys = sm_pool.tile([64, CH], F32, tag="ys", name=f"ys{c}_{hp}_{hl}")
                    nc.vector.tensor_copy(ys[:], pv[0:64, :])
                    ysrc = ys[:]
                else:
                    ysrc = pv[0:64, :]
                nc.vector.tensor_mul(
                    yT[hp][64 * hl:64 * (hl + 1), CH * c:CH * (c + 1)],
                    ysrc, bc[:])

        # ---- main schedule ----
        queue_proj(0)
        emit_fill(1 << 30)          # PROJ(0) back-to-back

        for c in range(N_CHUNKS):
            emit_fill(1 << 30)      # stragglers from the previous chunk
            if c + 2 < N_CHUNKS:
                load_x(c + 2)       # x prefetch, two chunks ahead
            if c + 1 < N_CHUNKS:
                queue_proj(c + 1)
            if c == 3:
                queue_p3(0)
                queue_p3(1)
                queue_p3(2, range(0, 4))
            nt = 4 * c + 4
            for hp in range(4):
                # spread remaining quanta over this chunk's remaining iters
                iters_left = (4 - hp) * nt
                emit_att_block(c, hp, pending[0] / iters_left)

        # tail: held-back out-projection work covers the last normalize
        # chain's latency before p3(chunk 3) can start
        queue_p3(2, range(4, 8), on_act=True)
        queue_p3(3, on_act=True)
        emit_fill(1 << 30)


# ---------------------------------------------------------------------------
# Host side
# ---------------------------------------------------------------------------

_NC_CACHE = None


def _get_nc():
    global _NC_CACHE
    if _NC_CACHE is None:
        _NC_CACHE = build_nc()
    return _NC_CACHE


def _host_prep(x, w_qkv, w_out, token_positions):
    """Build the 8 per-core input maps."""
    x = np.ascontiguousarray(np.asarray(x, dtype=np.float32))
    w_qkv = np.asarray(w_qkv, dtype=np.float32)
    w_out = np.asarray(w_out, dtype=np.float32)
    pos = np.asarray(token_positions).astype(np.float32)

    half = DH // 2
    inv_freq = (1.0 / (ROPE_THETA ** (np.arange(half, dtype=np.float32) * (2.0 / DH))))
    ang = pos[:, None] * inv_freq[None, :]          # [S, 32]
    cos = np.cos(ang).astype(np.float32)            # [S, 32]
    sin = np.sin(ang).astype(np.float32)
    # [Dh, S] interleaved-pair layout, duplicated for 2 heads per tile
    cos64 = np.repeat(cos.T, 2, axis=0)             # [64, S]
    sin64 = np.repeat(sin.T, 2, axis=0)
    sgn = np.where(np.arange(DH) % 2 == 0, -1.0, 1.0).astype(np.float32)
    sinp = sin64 * sgn[:, None]
    cosT = np.ascontiguousarray(np.tile(cos64, (2, 1)).astype(np.float16))  # [128, S]
    sinT = np.ascontiguousarray(np.tile(sinp, (2, 1)).astype(np.float16))

    tri = np.triu(np.ones((128, 128), dtype=np.float16))     # keep i <= j
    trimask = np.ascontiguousarray(np.concatenate([tri] * 4, axis=1))

    wq, wk, wv = w_qkv[0:D], w_qkv[D:2 * D], w_qkv[2 * D:3 * D]

    in_maps = []
    for core in range(N_CORES):
        b, g = divmod(core, 2)
        rows = slice(512 * g, 512 * (g + 1))
        wqkT = np.ascontiguousarray(
            np.concatenate([wq[rows], wk[rows]], axis=0).T.astype(np.float16))
        wvT = np.ascontiguousarray(wv[rows].T.astype(np.float16))
        woT = np.ascontiguousarray(w_out[:, rows].T.astype(np.float16))  # [512, 1024]
        xT = np.ascontiguousarray(x[b].T.astype(np.float16))
        in_maps.append({
            "xT": xT, "wqkT": wqkT, "wvT": wvT, "woT": woT,
            "cosT": cosT, "sinT": sinT, "trimask": trimask,
        })
    return in_maps


def _gather(results):
    out = np.empty((B, S, D), dtype=np.float32)
    for b in range(B):
        acc = results[2 * b]["outT"] + results[2 * b + 1]["outT"]   # [D, S]
        out[b] = acc.T
    return out


def kernel(x, w_qkv, w_out, token_positions, _trace=False, _trace_kwargs=None):
    nc = _get_nc()
    in_maps = _host_prep(x, w_qkv, w_out, token_positions)
    kw = {}
    if _trace:
        kw["trace"] = True
        kw.update(_trace_kwargs or {})
    res = run_bass_kernel_spmd(nc, in_maps, list(range(N_CORES)), **kw)
    out = _gather(res.results)
    if _trace:
        return out, res
    return out
